# revision 9
# baseline (speedup 1.0000x reference)
"""Causal attention kernel for Trainium2 (8 NeuronCores, SPMD over heads).

Problem: B=4, H=16, S=2048, D=64, fp32.
  scores = Q @ K^T / sqrt(64); causal mask; softmax (global-max shift in the
  reference cancels exactly); out = attn @ V.

Distribution: B*H = 64 heads -> 8 heads per core, embarrassingly parallel.

Per-core algorithm (per head, two q-passes of 1024):
  - QK: even k-tiles stream on PE row-tile T0 (partitions 0-63), odd on T8
    (64-127) -- measured concurrent on HW, ~2 moving-columns/cycle. K^T is
    packed into partition halves, Q^T duplicated into both halves.
  - exp is split across two engines running concurrently: ScalarE (exact
    exp, scale=1/8, plus a constant bias matching the DVE path's systematic
    relative bias) and the DVE via a custom 8-stage op:
    p = ((c0*z + c1)*z + c2)^16 == e^(z/8)*(1+eps), eps nearly constant,
    cancelling in the softmax ratio. Tile assignment balances the engines.
  - Causal diagonal block: GpSimd multiply by a triangular keep-mask.
  - PV: one 128-contraction matmul chain per k-tile into a single PSUM
    accumulator; [V|ones] gives the softmax denominator in row 64 for free.
  - PSUM evacuation split across ScalarE and DVE (GpSimd cannot read
    PSUM); DMA out^T (+rowsum row) as [65, S] per head; the host does the
    final divide-by-rowsum and transpose.
"""

import math
import os
import sys

import numpy as np

if "/opt/trn_rl_repo" not in sys.path:
    sys.path.insert(0, "/opt/trn_rl_repo")

B, H, S, D = 4, 16, 2048, 64
N_CORES = 8
HEADS_PER_CORE = (B * H) // N_CORES  # 8
PASS_Q = 1024  # q-columns per pass (2 PSUM banks)
CHUNK = 512  # PSUM bank boundary for fp32 outputs

# DVE exp: p = (C0*z + C1)*z + C2, squared 4x, z the raw 64-contraction
# Q.K score (exp arg z/8).  ScalarE path: exp(z*0.125 + BETA).
# Jointly optimized so both paths agree through the softmax ratio.
EXP_C0 = 3.4436267949839664e-05
EXP_C1 = 7.770817159682695e-03
EXP_C2 = 0.9999542988018534
EXP_BETA = -8.692886851909931e-04

_EXP_OP = [None]


def _register_exp_op():
    if _EXP_OP[0] is not None:
        return _EXP_OP[0]
    import concourse.dve_ops as dve_ops
    from concourse.dve_ops import DveOp
    from concourse.dve_spec import C0, C1, C2, Spec, Src0, sq

    def _ref(in0, in1, s0, s1, imm2):
        p = ((in0.astype(np.float32) * s0 + s1) * in0 + imm2).astype(np.float32)
        for _ in range(4):
            p = (p * p).astype(np.float32)
        return p

    op = DveOp(
        "EXP_PK16_ANT",
        Spec(body=sq(sq(sq(sq((Src0 * C0 + C1) * Src0 + C2)))), reference=_ref),
        subdim=False,
        uops_sha={"v3": "b9028a2770b985b4", "v4": "8a0143ec7033f2f1"},
    )
    if op.name not in dve_ops._SUB_OPCODE_FOR_NAME:
        dve_ops.OPS.append(op)
        dve_ops._SUB_OPCODE_FOR_NAME[op.name] = max(
            dve_ops._SUB_OPCODE_FOR_NAME.values()
        ) + 1
        dve_ops.CUSTOM_DVE_SPECS[op.name] = op.spec
    _EXP_OP[0] = op
    return op


def _chunks(lo, hi):
    """Split [lo, hi) at absolute multiples of CHUNK (PSUM bank boundaries)."""
    out = []
    c = lo
    while c < hi:
        w = min(hi, (c // CHUNK + 1) * CHUNK) - c
        out.append((c, w))
        c += w
    return out


def build_attention(tc, outs, ins, n_heads=HEADS_PER_CORE, s=S, pass_q=PASS_Q):
    import concourse.bass as bass
    import concourse.mybir as mybir

    exp_op = _register_exp_op()

    nc = tc.nc
    f32 = mybir.dt.float32
    f16 = mybir.dt.float16
    Exp = mybir.ActivationFunctionType.Exp

    qt_d, kt_d, v_d = ins["qt"], ins["kt"], ins["v"]
    tri_d = ins["ctri"]
    ot_d = outs["ot"]

    n_ktiles = s // 128
    n_pass = s // pass_q
    ktiles_per_pass = pass_q // 128

    with (
        tc.tile_pool(name="consts", bufs=1) as cpool,
        tc.tile_pool(name="qpool", bufs=3) as qpool,
        tc.tile_pool(name="kpool", bufs=3) as kpool,
        tc.tile_pool(name="vpool", bufs=3) as vpool,
        tc.tile_pool(name="atpool", bufs=6) as atpool,
        tc.tile_pool(name="osbpool", bufs=2) as osbpool,
        tc.tile_pool(name="scpool", bufs=2, space="PSUM") as scpool,
        tc.tile_pool(name="accpool", bufs=2, space="PSUM") as accApool,
    ):
        c_tri = cpool.tile([128, 128], f16, tag="ctri")
        nc.sync.dma_start(c_tri[:], tri_d[:])
        c_beta = cpool.tile([128, 1], f32, tag="cbeta")
        nc.sync.dma_start(c_beta[:], ins["cbeta"][:])

        for h in range(n_heads):
            # Q^T duplicated into both partition halves (row-tile packing).
            qt2 = qpool.tile([128, s], f16)
            nc.sync.dma_start(qt2[0:64, :], qt_d[h])
            nc.sync.dma_start(qt2[64:128, :], qt_d[h])
            # K^T: even k-tiles -> partitions 0-63, odd -> 64-127.
            kt2 = kpool.tile([128, s // 2], f16)
            kt_src = kt_d[h].rearrange("d (t two c) -> d two t c", two=2, c=128)
            kt2_v = kt2.rearrange("p (t c) -> p t c", c=128)
            nc.sync.dma_start(kt2_v[0:64], kt_src[:, 0])
            nc.sync.dma_start(kt2_v[64:128], kt_src[:, 1])
            # V with a ones-column pre-appended on the host: [128, n_ktiles, 65].
            vx = vpool.tile([128, n_ktiles * 65], f16)
            vx_v = vx.rearrange("p (t c) -> p t c", c=65)
            nc.sync.dma_start(vx_v[:], v_d[h].rearrange("(t p) d -> p t d", p=128))

            for p in range(n_pass):
                q0 = p * pass_q
                kmax = (p + 1) * ktiles_per_pass
                acc = accApool.tile([65, pass_q], f32, name=f"acc_{h}_{p}", tag="accA")
                pv_queue = []

                def _emit_pv(entries):
                    for (k, at, qlo) in entries:
                        for (c, w) in _chunks(qlo - q0, pass_q):
                            co = c - (qlo - q0)
                            nc.tensor.matmul(
                                acc[0:65, c : c + w],
                                vx_v[:, k, :],
                                at[:, co : co + w],
                                start=(k == 0),
                                stop=(k == kmax - 1),
                                skip_group_check=True,
                            )

                # exp engine assignment: balance ScalarE (0.833ns/col+185)
                # vs DVE (1.04ns/col+125): DVE takes odd k-tiles except the
                # two largest odd spans per pass go to ScalarE.
                for kp in range(0, kmax, 2):
                    pair = [k for k in (kp, kp + 1) if k < kmax]
                    scs, spans, qlos = {}, {}, {}
                    for k in pair:
                        qlos[k] = max(q0, 128 * k)
                        spans[k] = q0 + pass_q - qlos[k]
                        scs[k] = scpool.tile(
                            [128, pass_q], f32, tag="sc", name=f"sc_{h}_{p}_{k}"
                        )
                    # QK: interleave even/odd chunks -> T0/T8 concurrency
                    chunk_lists = {k: _chunks(0, spans[k]) for k in pair}
                    n_ch = max(len(v) for v in chunk_lists.values())
                    for ci in range(n_ch):
                        for k in pair:
                            if ci >= len(chunk_lists[k]):
                                continue
                            c, w = chunk_lists[k][ci]
                            half = k % 2
                            nc.tensor.matmul(
                                scs[k][:, c : c + w],
                                kt2_v[64 * half : 64 * half + 64, k // 2],
                                qt2[64 * half : 64 * half + 64,
                                    qlos[k] + c : qlos[k] + c + w],
                                start=True,
                                stop=True,
                                skip_group_check=True,
                            )
                    cur = []
                    for k in pair:
                        span, qlo = spans[k], qlos[k]
                        at = atpool.tile([128, pass_q], f16)
                        if k % 2 == 0:
                            nc.scalar.activation(
                                at[:, 0:span], scs[k][:, 0:span], Exp,
                                bias=c_beta[:, 0:1], scale=0.125,
                            )
                        else:
                            nc.vector._custom_dve(
                                exp_op,
                                out=at[:, 0:span],
                                in0=scs[k][:, 0:span],
                                s0=EXP_C0, s1=EXP_C1, imm2=EXP_C2,
                            )
                        if 128 * k >= q0:
                            # zero the masked upper part of the diagonal block
                            nc.gpsimd.tensor_mul(at[:, 0:128], at[:, 0:128], c_tri[:])
                        cur.append((k, at, qlo))
                    pv_queue.append(cur)
                    if len(pv_queue) > 1:
                        _emit_pv(pv_queue.pop(0))
                for entries in pv_queue:
                    _emit_pv(entries)
                # evacuate out^T (+rowsum row): half on ScalarE, half on
                # DVE (concurrent); DMA; host normalizes.
                osb = osbpool.tile([65, pass_q], f32, name=f"osb_{h}_{p}", tag="osb")
                nc.scalar.copy(osb[:, 0:512], acc[0:65, 0:512])
                nc.vector.tensor_copy(osb[:, 512:1024], acc[0:65, 512:1024])
                nc.sync.dma_start(ot_d[h, :, q0 : q0 + pass_q], osb[:])


def _make_consts():
    kk, qq = np.meshgrid(np.arange(128), np.arange(128), indexing="ij")
    tri = (kk <= qq).astype(np.float16)  # keep-mask for the diagonal block
    return tri


_NC_CACHE = {}


def _build_nc(n_heads=HEADS_PER_CORE, s=S, pass_q=PASS_Q):
    key = (n_heads, s, pass_q)
    if key in _NC_CACHE:
        return _NC_CACHE[key]
    import concourse.tile as tile
    from concourse import bacc, mybir

    nc = bacc.Bacc(
        "TRN2", target_bir_lowering=False, debug=False, enable_asserts=False
    )
    f32 = mybir.dt.float32
    f16 = mybir.dt.float16
    ins = {
        "qt": nc.dram_tensor("qt", [n_heads, D, s], f16, kind="ExternalInput").ap(),
        "kt": nc.dram_tensor("kt", [n_heads, D, s], f16, kind="ExternalInput").ap(),
        "v": nc.dram_tensor("v", [n_heads, s, D + 1], f16, kind="ExternalInput").ap(),
        "ctri": nc.dram_tensor("ctri", [128, 128], f16, kind="ExternalInput").ap(),
        "cbeta": nc.dram_tensor("cbeta", [128, 1], f32, kind="ExternalInput").ap(),
    }
    outs = {
        "ot": nc.dram_tensor("ot", [n_heads, 65, s], f32, kind="ExternalOutput").ap(),
    }
    with tile.TileContext(nc) as tc:
        build_attention(tc, outs, ins, n_heads=n_heads, s=s, pass_q=pass_q)
    nc.compile()
    _NC_CACHE[key] = nc
    return nc


def kernel(Q, K, V, mask, trace=False):
    """Full-input entry point: shards over 8 NeuronCores, returns full output."""
    from concourse.bass_utils import run_bass_kernel_spmd

    nc = _build_nc()
    tri = _make_consts()

    Qf = np.ascontiguousarray(
        Q.reshape(B * H, S, D).transpose(0, 2, 1), dtype=np.float16
    )
    Kf = np.ascontiguousarray(
        K.reshape(B * H, S, D).transpose(0, 2, 1), dtype=np.float16
    )
    Vf = np.concatenate(
        [
            V.reshape(B * H, S, D).astype(np.float16),
            np.ones((B * H, S, 1), dtype=np.float16),
        ],
        axis=-1,
    )

    in_maps = []
    for c in range(N_CORES):
        sl = slice(c * HEADS_PER_CORE, (c + 1) * HEADS_PER_CORE)
        in_maps.append(
            {
                "qt": Qf[sl],
                "kt": Kf[sl],
                "v": Vf[sl],
                "ctri": tri,
                "cbeta": np.full((128, 1), EXP_BETA, dtype=np.float32),
            }
        )

    res = run_bass_kernel_spmd(nc, in_maps, core_ids=list(range(N_CORES)), trace=trace)
    ot = np.concatenate([res.results[c]["ot"] for c in range(N_CORES)], axis=0)
    # ot: [B*H, 65, S] -- rows 0..63 are out^T columns, row 64 the rowsum.
    out = (ot[:, :64, :] / ot[:, 64:65, :]).transpose(0, 2, 1)
    out = out.reshape(B, H, S, D)
    kernel.last_results = res
    return np.ascontiguousarray(out, dtype=np.float32)


# revision 10
# speedup vs baseline: 1.1801x; 1.1801x over previous
"""Causal attention kernel for Trainium2 (8 NeuronCores, SPMD over heads).

Problem: B=4, H=16, S=2048, D=64, fp32.
  scores = Q @ K^T / sqrt(64); causal mask; softmax (global-max shift in the
  reference cancels exactly); out = attn @ V.

Distribution: B*H = 64 heads -> 8 heads per core, embarrassingly parallel.

Per-core algorithm (per head, two q-passes of 1024):
  - QK: even k-tiles stream on PE row-tile T0 (partitions 0-63), odd on T8
    (64-127) -- measured concurrent on HW, ~2 moving-columns/cycle. K^T is
    packed into partition halves, Q^T duplicated into both halves.
  - exp is split across two engines running concurrently: ScalarE (exact
    exp, scale=1/8, plus a constant bias matching the DVE path's systematic
    relative bias) and the DVE via a custom 8-stage op:
    p = ((c0*z + c1)*z + c2)^16 == e^(z/8)*(1+eps), eps nearly constant,
    cancelling in the softmax ratio. Tile assignment balances the engines.
  - Causal diagonal block: GpSimd multiply by a triangular keep-mask.
  - PV: split into k-row halves A (T0) and B (T8), 64-contraction each,
    accumulating into two PSUM tiles concurrently -- keeps the whole
    kernel in one (64,128) PE tiling mode (mode switches drain the PE).
    [V|ones] gives the softmax denominator in row 64 for free.
  - Merge/evacuation: ScalarE copies accB to SBUF, DVE adds accA (PSUM) +
    that copy; DMA out^T (+rowsum row) as [65, S] per head; the host does
    the final divide-by-rowsum and transpose.
"""

import math
import os
import sys

import numpy as np

if "/opt/trn_rl_repo" not in sys.path:
    sys.path.insert(0, "/opt/trn_rl_repo")

B, H, S, D = 4, 16, 2048, 64
N_CORES = 8
HEADS_PER_CORE = (B * H) // N_CORES  # 8
PASS_Q = 1024  # q-columns per pass (2 PSUM banks)
CHUNK = 512  # PSUM bank boundary for fp32 outputs

# DVE exp: p = (C0*z + C1)*z + C2, squared 4x, z the raw 64-contraction
# Q.K score (exp arg z/8).  ScalarE path: exp(z*0.125 + BETA).
# Jointly optimized so both paths agree through the softmax ratio.
EXP_C0 = 3.4436267949839664e-05
EXP_C1 = 7.770817159682695e-03
EXP_C2 = 0.9999542988018534
EXP_BETA = -8.692886851909931e-04

_EXP_OP = [None]


def _register_exp_op():
    if _EXP_OP[0] is not None:
        return _EXP_OP[0]
    import concourse.dve_ops as dve_ops
    from concourse.dve_ops import DveOp
    from concourse.dve_spec import C0, C1, C2, Spec, Src0, sq

    def _ref(in0, in1, s0, s1, imm2):
        p = ((in0.astype(np.float32) * s0 + s1) * in0 + imm2).astype(np.float32)
        for _ in range(4):
            p = (p * p).astype(np.float32)
        return p

    op = DveOp(
        "EXP_PK16_ANT",
        Spec(body=sq(sq(sq(sq((Src0 * C0 + C1) * Src0 + C2)))), reference=_ref),
        subdim=False,
        uops_sha={"v3": "b9028a2770b985b4", "v4": "8a0143ec7033f2f1"},
    )
    if op.name not in dve_ops._SUB_OPCODE_FOR_NAME:
        dve_ops.OPS.append(op)
        dve_ops._SUB_OPCODE_FOR_NAME[op.name] = max(
            dve_ops._SUB_OPCODE_FOR_NAME.values()
        ) + 1
        dve_ops.CUSTOM_DVE_SPECS[op.name] = op.spec
    _EXP_OP[0] = op
    return op


def _chunks(lo, hi):
    """Split [lo, hi) at absolute multiples of CHUNK (PSUM bank boundaries)."""
    out = []
    c = lo
    while c < hi:
        w = min(hi, (c // CHUNK + 1) * CHUNK) - c
        out.append((c, w))
        c += w
    return out


def build_attention(tc, outs, ins, n_heads=HEADS_PER_CORE, s=S, pass_q=PASS_Q):
    import concourse.bass as bass
    import concourse.mybir as mybir

    exp_op = _register_exp_op()

    nc = tc.nc
    f32 = mybir.dt.float32
    f16 = mybir.dt.float16
    Exp = mybir.ActivationFunctionType.Exp

    qt_d, kt_d, v_d = ins["qt"], ins["kt"], ins["v"]
    tri_d = ins["ctri"]
    ot_d = outs["ot"]

    n_ktiles = s // 128
    n_pass = s // pass_q
    ktiles_per_pass = pass_q // 128

    with (
        tc.tile_pool(name="consts", bufs=1) as cpool,
        tc.tile_pool(name="qpool", bufs=3) as qpool,
        tc.tile_pool(name="kpool", bufs=3) as kpool,
        tc.tile_pool(name="vpool", bufs=3) as vpool,
        tc.tile_pool(name="atpool", bufs=6) as atpool,
        tc.tile_pool(name="osbpool", bufs=2) as osbpool,
        tc.tile_pool(name="scpool", bufs=2, space="PSUM") as scpool,
        tc.tile_pool(name="accApool", bufs=1, space="PSUM") as accApool,
        tc.tile_pool(name="accBpool", bufs=1, space="PSUM") as accBpool,
    ):
        c_tri = cpool.tile([128, 128], f16, tag="ctri")
        nc.sync.dma_start(c_tri[:], tri_d[:])
        c_beta = cpool.tile([128, 1], f32, tag="cbeta")
        nc.sync.dma_start(c_beta[:], ins["cbeta"][:])

        for h in range(n_heads):
            # Q^T duplicated into both partition halves (row-tile packing).
            qt2 = qpool.tile([128, s], f16)
            nc.sync.dma_start(qt2[0:64, :], qt_d[h])
            nc.sync.dma_start(qt2[64:128, :], qt_d[h])
            # K^T: even k-tiles -> partitions 0-63, odd -> 64-127.
            kt2 = kpool.tile([128, s // 2], f16)
            kt_src = kt_d[h].rearrange("d (t two c) -> d two t c", two=2, c=128)
            kt2_v = kt2.rearrange("p (t c) -> p t c", c=128)
            nc.sync.dma_start(kt2_v[0:64], kt_src[:, 0])
            nc.sync.dma_start(kt2_v[64:128], kt_src[:, 1])
            # V with a ones-column pre-appended on the host: [128, n_ktiles, 65].
            vx = vpool.tile([128, n_ktiles * 65], f16)
            vx_v = vx.rearrange("p (t c) -> p t c", c=65)
            nc.sync.dma_start(vx_v[:], v_d[h].rearrange("(t p) d -> p t d", p=128))

            for p in range(n_pass):
                q0 = p * pass_q
                kmax = (p + 1) * ktiles_per_pass
                accA = accApool.tile([65, pass_q], f32, name=f"accA_{h}_{p}", tag="accA")
                accB = accBpool.tile([65, pass_q], f32, name=f"accB_{h}_{p}", tag="accB")
                pv_queue = []

                def _emit_pv(entries):
                    for (k, at, qlo) in entries:
                        for (c, w) in _chunks(qlo - q0, pass_q):
                            co = c - (qlo - q0)
                            nc.tensor.matmul(
                                accA[0:65, c : c + w],
                                vx_v[0:64, k, :],
                                at[0:64, co : co + w],
                                start=(k == 0),
                                stop=(k == kmax - 1),
                                skip_group_check=True,
                            )
                            nc.tensor.matmul(
                                accB[0:65, c : c + w],
                                vx_v[64:128, k, :],
                                at[64:128, co : co + w],
                                start=(k == 0),
                                stop=(k == kmax - 1),
                                skip_group_check=True,
                            )

                # exp engine assignment: balance ScalarE (0.833ns/col+185)
                # vs DVE (1.04ns/col+125): DVE takes odd k-tiles except the
                # two largest odd spans per pass go to ScalarE.
                for kp in range(0, kmax, 2):
                    pair = [k for k in (kp, kp + 1) if k < kmax]
                    scs, spans, qlos = {}, {}, {}
                    for k in pair:
                        qlos[k] = max(q0, 128 * k)
                        spans[k] = q0 + pass_q - qlos[k]
                        scs[k] = scpool.tile(
                            [128, pass_q], f32, tag="sc", name=f"sc_{h}_{p}_{k}"
                        )
                    # QK: interleave even/odd chunks -> T0/T8 concurrency
                    chunk_lists = {k: _chunks(0, spans[k]) for k in pair}
                    n_ch = max(len(v) for v in chunk_lists.values())
                    for ci in range(n_ch):
                        for k in pair:
                            if ci >= len(chunk_lists[k]):
                                continue
                            c, w = chunk_lists[k][ci]
                            half = k % 2
                            nc.tensor.matmul(
                                scs[k][:, c : c + w],
                                kt2_v[64 * half : 64 * half + 64, k // 2],
                                qt2[64 * half : 64 * half + 64,
                                    qlos[k] + c : qlos[k] + c + w],
                                start=True,
                                stop=True,
                                skip_group_check=True,
                            )
                    cur = []
                    for k in pair:
                        span, qlo = spans[k], qlos[k]
                        at = atpool.tile([128, pass_q], f16)
                        if k % 2 == 0:
                            nc.scalar.activation(
                                at[:, 0:span], scs[k][:, 0:span], Exp,
                                bias=c_beta[:, 0:1], scale=0.125,
                            )
                        else:
                            nc.vector._custom_dve(
                                exp_op,
                                out=at[:, 0:span],
                                in0=scs[k][:, 0:span],
                                s0=EXP_C0, s1=EXP_C1, imm2=EXP_C2,
                            )
                        if 128 * k >= q0:
                            # zero the masked upper part of the diagonal block
                            nc.gpsimd.tensor_mul(at[:, 0:128], at[:, 0:128], c_tri[:])
                        cur.append((k, at, qlo))
                    pv_queue.append(cur)
                    if len(pv_queue) > 1:
                        _emit_pv(pv_queue.pop(0))
                for entries in pv_queue:
                    _emit_pv(entries)
                # merge + evacuate out^T (+rowsum row): ScalarE copies accB
                # to SBUF, DVE adds accA; DMA; host normalizes.
                osbB = osbpool.tile([65, pass_q], f32, name=f"osbB_{h}_{p}", tag="osbB")
                nc.scalar.copy(osbB[:], accB[0:65, :])
                osb = osbpool.tile([65, pass_q], f32, name=f"osb_{h}_{p}", tag="osb")
                nc.vector.tensor_add(osb[:], accA[0:65, :], osbB[:])
                nc.sync.dma_start(ot_d[h, :, q0 : q0 + pass_q], osb[:])


def _make_consts():
    kk, qq = np.meshgrid(np.arange(128), np.arange(128), indexing="ij")
    tri = (kk <= qq).astype(np.float16)  # keep-mask for the diagonal block
    return tri


_NC_CACHE = {}


def _build_nc(n_heads=HEADS_PER_CORE, s=S, pass_q=PASS_Q):
    key = (n_heads, s, pass_q)
    if key in _NC_CACHE:
        return _NC_CACHE[key]
    import concourse.tile as tile
    from concourse import bacc, mybir

    nc = bacc.Bacc(
        "TRN2", target_bir_lowering=False, debug=False, enable_asserts=False
    )
    f32 = mybir.dt.float32
    f16 = mybir.dt.float16
    ins = {
        "qt": nc.dram_tensor("qt", [n_heads, D, s], f16, kind="ExternalInput").ap(),
        "kt": nc.dram_tensor("kt", [n_heads, D, s], f16, kind="ExternalInput").ap(),
        "v": nc.dram_tensor("v", [n_heads, s, D + 1], f16, kind="ExternalInput").ap(),
        "ctri": nc.dram_tensor("ctri", [128, 128], f16, kind="ExternalInput").ap(),
        "cbeta": nc.dram_tensor("cbeta", [128, 1], f32, kind="ExternalInput").ap(),
    }
    outs = {
        "ot": nc.dram_tensor("ot", [n_heads, 65, s], f32, kind="ExternalOutput").ap(),
    }
    with tile.TileContext(nc) as tc:
        build_attention(tc, outs, ins, n_heads=n_heads, s=s, pass_q=pass_q)
    nc.compile()
    _NC_CACHE[key] = nc
    return nc


def kernel(Q, K, V, mask, trace=False):
    """Full-input entry point: shards over 8 NeuronCores, returns full output."""
    from concourse.bass_utils import run_bass_kernel_spmd

    nc = _build_nc()
    tri = _make_consts()

    Qf = np.ascontiguousarray(
        Q.reshape(B * H, S, D).transpose(0, 2, 1), dtype=np.float16
    )
    Kf = np.ascontiguousarray(
        K.reshape(B * H, S, D).transpose(0, 2, 1), dtype=np.float16
    )
    Vf = np.concatenate(
        [
            V.reshape(B * H, S, D).astype(np.float16),
            np.ones((B * H, S, 1), dtype=np.float16),
        ],
        axis=-1,
    )

    in_maps = []
    for c in range(N_CORES):
        sl = slice(c * HEADS_PER_CORE, (c + 1) * HEADS_PER_CORE)
        in_maps.append(
            {
                "qt": Qf[sl],
                "kt": Kf[sl],
                "v": Vf[sl],
                "ctri": tri,
                "cbeta": np.full((128, 1), EXP_BETA, dtype=np.float32),
            }
        )

    res = run_bass_kernel_spmd(nc, in_maps, core_ids=list(range(N_CORES)), trace=trace)
    ot = np.concatenate([res.results[c]["ot"] for c in range(N_CORES)], axis=0)
    # ot: [B*H, 65, S] -- rows 0..63 are out^T columns, row 64 the rowsum.
    out = (ot[:, :64, :] / ot[:, 64:65, :]).transpose(0, 2, 1)
    out = out.reshape(B, H, S, D)
    kernel.last_results = res
    return np.ascontiguousarray(out, dtype=np.float32)
